# revision 1
# baseline (speedup 1.0000x reference)
"""Trainium2 Bass kernel for GCN(3-layer) + BiLSTM(2-layer) + FC.

Self-contained: hardcodes all shapes; needs /opt/trn_rl_repo (concourse) only.

Architecture (8 NeuronCores, SPMD):
 - Nodes sharded by dst range (40960/core, graph-aligned).  A_hat = D^-1/2 (A+I) D^-1/2.
 - GCN layer k: gather messages m_k[src] (per-128-row indirect DMA, in
   dst-cell order), scatter-add via PE matmuls against host-built sparse cell
   blocks S (norm weights folded in), feature-major psum per 512-node window,
   fused bias/relu on ACT, next-layer premultiply m_{k+1} = h_k @ W on PE,
   AllGather of the m shards between layers.
 - BiLSTM: batch-sharded 16 seq/core, feature-major state (H=128 partitions),
   per-step gate matmuls on PE, sigmoid/tanh on ACT, cell math on DVE.
"""
import sys, os
sys.path.insert(0, "/opt/trn_rl_repo")
import numpy as np
import ml_dtypes

import concourse.bass as bass
import concourse.bacc as bacc
import concourse.tile as tile
from concourse import mybir
from concourse.bass_utils import run_bass_kernel_spmd
from concourse.vector_clock import ScopedClock

BF16 = ml_dtypes.bfloat16
AF = mybir.ActivationFunctionType

# ---- problem constants (hardcoded) ----
B, T, FEAT, H, GCN_H, NCLS = 128, 512, 320, 128, 32, 10
N = B * 5 * T            # 327680
FIN = FEAT // 5          # 64
LSTM_IN = 5 * GCN_H      # 160
NCORES = 8
NS = N // NCORES         # 40960 nodes/core
WIN = 512                # window cols
NW = NS // WIN           # 80 windows/core
CELL = 12
NCELL = 43               # 42*12 + 8 = 512
MAXE = 128               # max edges per cell (verified on data: 126)
BC = B // NCORES         # 16 sequences per core
TB = T * BC              # 8192 token-cols per core
XWCH = 16                # lstm xw prefetch chunk (steps)


class TC(tile.TileContext):
    """TileContext whose tail drain splits its sem waits across multiple SP
    instructions (walrus rejects >1 wait condition on an InstDrain)."""
    def _drain_and_barrier(self, tick_clock, wait_clock):
        drain_inst = self.nc.sync.drain()
        wait_clock.add_sem_waits(
            drain_inst.ins, ScopedClock({None: tick_clock.global_clock})
        )
        si = drain_inst.ins.sync_info
        if si is not None and si.on_wait is not None:
            waits = list(si.on_wait)
            if len(waits) > 1:
                si.on_wait = waits[:1]
                for i in range(1, len(waits)):
                    extra = self.nc.sync.drain()
                    extra.ins.sync_info = mybir.SyncInfo(
                        on_wait=waits[i:i+1], on_update=[])
        self.nc.all_engine_barrier()
        assert self.sems is not None
        popped = self.nc._tile_sem_poison_stack.pop()
        assert popped is self._sem_poison
        self.nc.clear_and_free_semaphores(list(self.sems.allocated().values()))
        self.nc.all_engine_barrier()


# =====================================================================
# Host preprocessing
# =====================================================================

def _prep_graph(edge_src, edge_dst):
    src = np.asarray(edge_src, np.int64)
    dst = np.asarray(edge_dst, np.int64)
    deg = np.bincount(dst, minlength=N).astype(np.float64) + 1.0
    dinv = 1.0 / np.sqrt(deg)
    sl = np.arange(N, dtype=np.int64)
    s_all = np.concatenate([src, sl])
    d_all = np.concatenate([dst, sl])
    w_all = (dinv[s_all] * dinv[d_all]).astype(np.float32)

    # layers 2/3 gather-index remap: m rows are stored (window, p, c) packed
    vv = np.arange(N, dtype=np.int64)
    loc = vv % WIN
    M_REMAP = (vv // WIN) * WIN + (loc % 128) * 4 + loc // 128
    idx_cores, S_cores = [], []
    for c in range(NCORES):
        m = (d_all // NS) == c
        s_c = s_all[m]; d_c = d_all[m] - c * NS; w_c = w_all[m]
        wid = d_c // WIN
        cj = (d_c % WIN) // CELL
        cell_id = wid * NCELL + cj
        order = np.argsort(cell_id, kind="stable")
        s_c = s_c[order]; d_c = d_c[order]; w_c = w_c[order]
        cell_id = cell_id[order]
        counts = np.bincount(cell_id, minlength=NW * NCELL)
        assert counts.max() <= MAXE, counts.max()
        starts = np.concatenate([[0], np.cumsum(counts)])
        slot = np.arange(len(s_c)) - starts[cell_id]
        wi = cell_id // NCELL
        ji = cell_id % NCELL
        idx_arr = np.zeros((NW, NCELL, MAXE), np.int32)
        idx_arr[wi, ji, slot] = s_c
        # S: (MAXE, NW*512); cell j of window w at cols [w*512 + j*12, +cw)
        S_dev = np.zeros((MAXE, NW * WIN), np.float32)
        col = (wi * WIN + ji * CELL) + (d_c - wi * WIN - ji * CELL)
        S_dev[slot, col] = w_c
        i1 = np.ascontiguousarray(idx_arr.transpose(2, 0, 1).reshape(MAXE, NW * NCELL))
        i23 = M_REMAP[i1].astype(np.int32)
        idx_cores.append((i1, i23))
        S_cores.append(S_dev.astype(BF16))
    return idx_cores, S_cores


def _prep_weights(inp):
    d = {}
    d["W1"] = np.asarray(inp["W1"], np.float32).astype(BF16)
    d["W2"] = np.asarray(inp["W2"], np.float32).astype(BF16)
    d["W3"] = np.asarray(inp["W3"], np.float32).astype(BF16)
    for k in ("b1", "b2", "b3"):
        d[k] = np.asarray(inp[k], np.float32).reshape(GCN_H, 1)

    def pack_ih(Wih, bih, bhh):
        Wih = np.asarray(Wih, np.float32)
        inn = Wih.shape[1]
        o = np.zeros((inn + 1, 4 * H), np.float32)
        o[:inn] = Wih.T
        o[inn] = np.asarray(bih, np.float32) + np.asarray(bhh, np.float32)
        return o.astype(BF16)

    def pack_hh(Whh):
        Whh = np.asarray(Whh, np.float32)
        return np.concatenate(
            [Whh[g*H:(g+1)*H, :].T for g in range(4)], axis=1).astype(BF16)

    for tag in ("0", "1"):
        for dr in ("f", "b"):
            d[f"wih{tag}{dr}"] = pack_ih(
                inp[f"Wih{tag}{dr}"], inp[f"bih{tag}{dr}"], inp[f"bhh{tag}{dr}"])
            d[f"whh{tag}{dr}"] = pack_hh(inp[f"Whh{tag}{dr}"])
    d["wfc"] = np.asarray(inp["Wfc"], np.float32).astype(BF16)
    d["bfc"] = np.broadcast_to(
        np.asarray(inp["bfc"], np.float32), (BC, NCLS)).copy()
    return d


# =====================================================================
# Bass kernel builder
# =====================================================================

# torch gate order i,f,g,o -> device col blocks [i f o g]
GBLK = {0: 0, 1: 1, 2: 3, 3: 2}


def build_kernel():
    nc = bacc.Bacc(None, num_devices=NCORES)
    dt = mybir.dt
    f32, bf16, i32 = dt.float32, dt.bfloat16, dt.int32

    xt = nc.dram_tensor("xt", [N, FIN], bf16, kind="ExternalInput")
    idxT = nc.dram_tensor("idxT", [MAXE, NW * NCELL], i32, kind="ExternalInput")
    idxT2 = nc.dram_tensor("idxT2", [MAXE, NW * NCELL], i32, kind="ExternalInput")
    ST = nc.dram_tensor("ST", [MAXE, NW * WIN], bf16, kind="ExternalInput")
    W1 = nc.dram_tensor("W1", [FIN, GCN_H], bf16, kind="ExternalInput")
    W2 = nc.dram_tensor("W2", [GCN_H, GCN_H], bf16, kind="ExternalInput")
    W3 = nc.dram_tensor("W3", [GCN_H, GCN_H], bf16, kind="ExternalInput")
    b1 = nc.dram_tensor("b1", [GCN_H, 1], f32, kind="ExternalInput")
    b2 = nc.dram_tensor("b2", [GCN_H, 1], f32, kind="ExternalInput")
    b3 = nc.dram_tensor("b3", [GCN_H, 1], f32, kind="ExternalInput")
    wih, whh = {}, {}
    for tag, inn in (("0", LSTM_IN), ("1", 2 * H)):
        for dr in ("f", "b"):
            wih[tag+dr] = nc.dram_tensor(f"wih{tag}{dr}", [inn + 1, 4 * H], bf16, kind="ExternalInput")
            whh[tag+dr] = nc.dram_tensor(f"whh{tag}{dr}", [H, 4 * H], bf16, kind="ExternalInput")
    wfc = nc.dram_tensor("wfc", [2 * H, NCLS], bf16, kind="ExternalInput")
    bfc = nc.dram_tensor("bfc", [BC, NCLS], f32, kind="ExternalInput")
    out = nc.dram_tensor("out", [BC, NCLS], f32, kind="ExternalOutput")

    m2s = nc.dram_tensor("m2s", [NS, GCN_H], bf16)
    m3s = nc.dram_tensor("m3s", [NS, GCN_H], bf16)
    m2f = nc.dram_tensor("m2f", [N, GCN_H], bf16, addr_space="Shared")
    m3f = nc.dram_tensor("m3f", [N, GCN_H], bf16, addr_space="Shared")
    # xw tensors: per phase, per dir: (H, T*4*BC): col = t*64 + blk*16 + b
    xwd = {ph+dr: nc.dram_tensor(f"xw{ph}{dr}", [H, T * 4 * BC], bf16)
           for ph in ("0", "1") for dr in ("f", "b")}

    RG = [list(range(NCORES))]

    with tile.TileContext(nc) as tc:
        with tc.tile_pool(name="persist", bufs=1) as pp:
            w1_sb = pp.tile([FIN, GCN_H], bf16)
            nc.gpsimd.dma_start(w1_sb[:], W1[:])
            w2_sb = pp.tile([GCN_H, GCN_H], bf16)
            nc.gpsimd.dma_start(w2_sb[:], W2[:])
            w3_sb = pp.tile([GCN_H, GCN_H], bf16)
            nc.gpsimd.dma_start(w3_sb[:], W3[:])
            b_sb = {}
            for nm, t_ in (("b1", b1), ("b2", b2), ("b3", b3)):
                bias_tile = pp.tile([GCN_H, 1], f32, tag=nm)
                bias_tile = bias_tile; b_sb[nm] = bias_tile
                nc.gpsimd.dma_start(b_sb[nm][:], t_[:])
            gcn3T = pp.tile([GCN_H, NS], bf16)

            # ---------------- GCN ----------------
            def gcn_layer(lay, table, dst_shard, S_sb=None, idx_all=None):
                pass
                DIN = FIN if lay == 1 else GCN_H
                wnext = {1: w2_sb, 2: w3_sb}.get(lay)
                bias = b_sb[f"b{lay}"]
                with tc.tile_pool(name=f"gcn{lay}", bufs=4) as gp, \
                     tc.tile_pool(name=f"gcnp{lay}", bufs=2, space="PSUM") as gps, \
                     tc.tile_pool(name=f"gcns{lay}", bufs=2) as sp2:
                    for w in range(NW):
                        agg = gps.tile([DIN, WIN], f32, tag="agg")
                        for j in range(NCELL):
                            cw = min(CELL, WIN - j * CELL)
                            g = gp.tile([MAXE, DIN], bf16, tag="g")
                            nc.gpsimd.indirect_dma_start(
                                out=g[:], out_offset=None, in_=table[:],
                                in_offset=bass.IndirectOffsetOnAxis(
                                    ap=idx_all[:, w*NCELL+j : w*NCELL+j+1], axis=0),
                            )
                            nc.tensor.matmul(
                                agg[:, j*CELL:j*CELL+cw], lhsT=g[:],
                                rhs=S_sb[:, w*WIN + j*CELL: w*WIN + j*CELL + cw],
                                start=True, stop=True)
                        if lay == 1:
                            aggs = sp2.tile([FIN, WIN], bf16, tag="aggs")
                            nc.scalar.copy(aggs[:], agg[:])
                            h_ps = gps.tile([GCN_H, WIN], f32, tag="hps")
                            nc.tensor.matmul(h_ps[:], lhsT=w1_sb[:], rhs=aggs[:],
                                             start=True, stop=True)
                            hT = sp2.tile([GCN_H, WIN], bf16, tag="hT")
                            nc.scalar.activation(hT[:], h_ps[:], AF.Relu, bias=bias[:])
                        elif lay == 2:
                            hT = sp2.tile([GCN_H, WIN], bf16, tag="hT")
                            nc.scalar.activation(hT[:], agg[:], AF.Relu, bias=bias[:])
                        else:
                            nc.scalar.activation(gcn3T[:, w*WIN:(w+1)*WIN], agg[:],
                                                 AF.Identity, bias=bias[:])
                        if dst_shard is not None:
                            mn_ps = gps.tile([MAXE, 4 * GCN_H], f32, tag="mnps")
                            for cc in range(4):
                                nc.tensor.matmul(
                                    mn_ps[:, cc*GCN_H:(cc+1)*GCN_H],
                                    lhsT=hT[:, cc*128:(cc+1)*128],
                                    rhs=wnext[:], start=True, stop=True)
                            mn = sp2.tile([MAXE, 4 * GCN_H], bf16, tag="mn")
                            nc.vector.tensor_copy(mn[:], mn_ps[:])
                            nc.gpsimd.dma_start(
                                dst_shard.ap().rearrange(
                                    "(w p c) f -> w p (c f)", w=NW, p=MAXE)[w],
                                mn[:])

            with tc.tile_pool(name="spool", bufs=1) as spool:
                S_sb = spool.tile([MAXE, NW * WIN], bf16)
                nc.gpsimd.dma_start(S_sb[:], ST[:])
                idx_all = spool.tile([MAXE, NW * NCELL], i32)
                nc.gpsimd.dma_start(idx_all[:], idxT[:])
                idx_all2 = spool.tile([MAXE, NW * NCELL], i32)
                nc.gpsimd.dma_start(idx_all2[:], idxT2[:])
                gcn_layer(1, xt, m2s, S_sb, idx_all)
                nc.gpsimd.collective_compute(
                    "AllGather", mybir.AluOpType.bypass, replica_groups=RG,
                    ins=[m2s.ap().opt()], outs=[m2f.ap().opt()])
                gcn_layer(2, m2f, m3s, S_sb, idx_all2)
                nc.gpsimd.collective_compute(
                    "AllGather", mybir.AluOpType.bypass, replica_groups=RG,
                    ins=[m3s.ap().opt()], outs=[m3f.ap().opt()])
                gcn_layer(3, m3f, None, S_sb, idx_all2)

            # ---------------- LSTM ----------------
            with tc.tile_pool(name="lstm", bufs=1) as lp:
                tokA = lp.tile([H, TB], bf16)
                tokB = lp.tile([32, TB], bf16)
                # tokens col order t*BC + b; gcn3T col = b*2560 + 5t + u
                for u in range(5):
                    dst = (tokA[32*u:32*(u+1), :] if u < 4 else tokB[:, :])
                    src = gcn3T[:].rearrange("f (b t u) -> f u t b", b=BC, u=5)[:, u]
                    nc.vector.tensor_copy(
                        dst.rearrange("f (t b) -> f t b", b=BC), src)
                
                h0f = lp.tile([H, TB], bf16)
                h0b = lp.tile([H, TB], bf16)
                h1f = lp.tile([H, TB], bf16)
                h1b = lp.tile([H, TB], bf16)
                ones1 = lp.tile([1, WIN], bf16)
                nc.vector.memset(ones1[:], 1.0)
                zeroBC = lp.tile([H, BC], bf16)
                nc.vector.memset(zeroBC[:], 0.0)

                wih_sb = {}
                for key, inn in (("0f", LSTM_IN), ("0b", LSTM_IN),
                                 ("1f", 2*H), ("1b", 2*H)):
                    bspan = inn - 128
                    wihA = lp.tile([128, 4 * H], bf16, tag=f"wihA{key}")
                    nc.gpsimd.dma_start(wihA[:], wih[key][0:128])
                    wihB = lp.tile([bspan, 4 * H], bf16, tag=f"wihB{key}")
                    nc.gpsimd.dma_start(wihB[:], wih[key][128:inn])
                    wihBias = lp.tile([1, 4 * H], bf16, tag=f"wihC{key}")
                    nc.gpsimd.dma_start(wihBias[:], wih[key][inn:inn+1])
                    wih_sb[key] = (wihA, wihB, wihBias)
                whh_sb = {}
                for key in ("0f", "0b", "1f", "1b"):
                    whhT = lp.tile([H, 4 * H], bf16, tag=f"whh{key}")
                    nc.gpsimd.dma_start(whhT[:], whh[key][:])
                    whh_sb[key] = whhT

                def xw_precompute(ph, chunks_f, chunks_b):
                    # chunks: per dir list of (wih_rows_slice, rhs_ap) K-chunks
                    with tc.tile_pool(name=f"xw{ph}", bufs=3) as xp, \
                         tc.tile_pool(name=f"xwp{ph}", bufs=2, space="PSUM") as xps:
                        for dr, chunks in (("f", chunks_f), ("b", chunks_b)):
                            for G in range(4):
                                blk = GBLK[G]
                                for ct in range(TB // WIN):
                                    ps = xps.tile([H, WIN], f32, tag="ps")
                                    nchunks = len(chunks)
                                    for ci, (lhsT, rhs) in enumerate(chunks):
                                        rr = (rhs[:, 0:WIN] if rhs.shape[0] == 1
                                              else rhs[:, ct*WIN:(ct+1)*WIN])
                                        nc.tensor.matmul(
                                            ps[:], lhsT=lhsT[:, G*H:(G+1)*H],
                                            rhs=rr,
                                            start=(ci == 0), stop=(ci == nchunks-1))
                                    sb = xp.tile([H, WIN], bf16, tag="sb")
                                    nc.vector.tensor_copy(sb[:], ps[:])
                                    # layout (g, t, b): gate block contiguous
                                    nc.gpsimd.dma_start(
                                        xwd[ph+dr].ap()[:, blk*TB + ct*WIN: blk*TB + (ct+1)*WIN],
                                        sb[:])

                def lstm_phase(ph, hf_st, hb_st):
                    with tc.tile_pool(name=f"lph{ph}", bufs=2) as php, \
                         tc.tile_pool(name=f"lps{ph}", bufs=4, space="PSUM") as phps:
                        cboth = php.tile([H, 2 * BC], f32, tag="cboth")
                        nc.vector.memset(cboth[:], 0.0)
                        xwf_b = xwb_b = None
                        for t in range(T):
                            tt = T - 1 - t
                            if t % XWCH == 0:
                                xwf_b = php.tile([H, 4, XWCH * BC], bf16, tag="xwfb")
                                xwb_b = php.tile([H, 4, XWCH * BC], bf16, tag="xwbb")
                                for Gb in range(4):
                                    nc.gpsimd.dma_start(
                                        xwf_b[:, Gb, :],
                                        xwd[ph+"f"].ap()[:, Gb*TB + t*BC: Gb*TB + (t+XWCH)*BC])
                                    nc.gpsimd.dma_start(
                                        xwb_b[:, Gb, :],
                                        xwd[ph+"b"].ap()[:, Gb*TB + (tt-XWCH+1)*BC: Gb*TB + (tt+1)*BC])
                            of = (t % XWCH) * BC
                            ob = (XWCH - 1 - (t % XWCH)) * BC
                            gps_t = phps.tile([H, 8 * BC], f32, tag="gates")
                            for d_i, dr in ((0, "f"), (1, "b")):
                                st = hf_st if d_i == 0 else hb_st
                                tm = t if d_i == 0 else tt
                                if t == 0:
                                    hprev = zeroBC[:]
                                elif d_i == 0:
                                    hprev = st[:, (tm-1)*BC:tm*BC]
                                else:
                                    hprev = st[:, (tm+1)*BC:(tm+2)*BC]
                                wt = whh_sb[ph + dr]
                                for G in range(4):
                                    blk = GBLK[G]
                                    nc.tensor.matmul(
                                        gps_t[:, blk*2*BC + d_i*BC: blk*2*BC + (d_i+1)*BC],
                                        lhsT=wt[:, G*H:(G+1)*H], rhs=hprev,
                                        start=True, stop=True)
                            gf = php.tile([H, 8 * BC], f32, tag="gf")
                            g4 = gps_t[:].rearrange("h (g d b) -> h g d b", g=4, d=2)
                            gf4 = gf[:].rearrange("h (g d b) -> h g d b", g=4, d=2)
                            nc.vector.tensor_add(
                                gf4[:, :, 0, :], g4[:, :, 0, :],
                                xwf_b[:, :, of:of+BC])
                            nc.vector.tensor_add(
                                gf4[:, :, 1, :], g4[:, :, 1, :],
                                xwb_b[:, :, ob:ob+BC])
                            sact = php.tile([H, 8 * BC], f32, tag="sact")
                            nc.scalar.activation(sact[:, 0:6*BC], gf[:, 0:6*BC], AF.Sigmoid)
                            nc.scalar.activation(sact[:, 6*BC:8*BC], gf[:, 6*BC:8*BC], AF.Tanh)
                            mm = php.tile([H, 2 * BC], f32, tag="mm")
                            nc.vector.tensor_mul(mm[:], sact[:, 0:2*BC], sact[:, 6*BC:8*BC])
                            ctm = php.tile([H, 2 * BC], f32, tag="ctm")
                            nc.vector.tensor_mul(ctm[:], sact[:, 2*BC:4*BC], cboth[:])
                            nc.vector.tensor_add(cboth[:], ctm[:], mm[:])
                            tct = php.tile([H, 2 * BC], f32, tag="tct")
                            nc.scalar.activation(tct[:], cboth[:], AF.Tanh)
                            nc.vector.tensor_mul(
                                hf_st[:, t*BC:(t+1)*BC], sact[:, 4*BC:5*BC], tct[:, 0:BC])
                            nc.vector.tensor_mul(
                                hb_st[:, tt*BC:(tt+1)*BC], sact[:, 5*BC:6*BC], tct[:, BC:])

                def chunks_for(key, rA, rB):
                    a, bwt, cbias = wih_sb[key]
                    return [(a, rA), (bwt, rB), (cbias, ones1)]
                xw_precompute("0", chunks_for("0f", tokA, tokB),
                              chunks_for("0b", tokA, tokB))
                lstm_phase("0", h0f, h0b)
                xw_precompute("1", chunks_for("1f", h0f, h0b),
                              chunks_for("1b", h0f, h0b))
                lstm_phase("1", h1f, h1b)

                wfc_a = lp.tile([H, NCLS], bf16)
                nc.gpsimd.dma_start(wfc_a[:], wfc[0:H])
                wfc_b = lp.tile([H, NCLS], bf16)
                nc.gpsimd.dma_start(wfc_b[:], wfc[H:])
                bfc_sb = lp.tile([BC, NCLS], f32)
                nc.gpsimd.dma_start(bfc_sb[:], bfc[:])
                with tc.tile_pool(name="fcps", bufs=1, space="PSUM") as fps:
                    fc_ps = fps.tile([BC, NCLS], f32)
                    nc.tensor.matmul(fc_ps[:], lhsT=h1f[:, (T-1)*BC:T*BC],
                                     rhs=wfc_a[:], start=True, stop=False)
                    nc.tensor.matmul(fc_ps[:], lhsT=h1b[:, (T-1)*BC:T*BC],
                                     rhs=wfc_b[:], start=False, stop=True)
                    fc_sb = lp.tile([BC, NCLS], f32)
                    nc.vector.tensor_add(fc_sb[:], fc_ps[:], bfc_sb[:])
                    nc.gpsimd.dma_start(out[:], fc_sb[:])
    return nc


# =====================================================================
# Entry
# =====================================================================

_CACHED = {}

def kernel(**inputs):
    x = np.asarray(inputs["x"], np.float32)
    idx_cores, S_cores = _prep_graph(inputs["edge_src"], inputs["edge_dst"])
    wts = _prep_weights(inputs)
    xt_full = x.astype(BF16)

    if "nc" not in _CACHED:
        nc_new = build_kernel()
        if not nc_new.is_finalized():
            nc_new.finalize()
        _CACHED["nc"] = nc_new
    nc = _CACHED["nc"]

    in_maps = []
    for c in range(NCORES):
        m = dict(
            xt=xt_full, idxT=idx_cores[c][0], idxT2=idx_cores[c][1], ST=S_cores[c],
            W1=wts["W1"], W2=wts["W2"], W3=wts["W3"],
            b1=wts["b1"], b2=wts["b2"], b3=wts["b3"],
            wfc=wts["wfc"], bfc=wts["bfc"],
        )
        for tag in ("0", "1"):
            for dr in ("f", "b"):
                m[f"wih{tag}{dr}"] = wts[f"wih{tag}{dr}"]
                m[f"whh{tag}{dr}"] = wts[f"whh{tag}{dr}"]
        in_maps.append(m)

    res = run_bass_kernel_spmd(nc, in_maps, core_ids=list(range(NCORES)),
                               trace=os.environ.get("KTRACE", "0") == "1")
    kernel.last_result = res
    outs = [res.results[c]["out"] for c in range(NCORES)]
    return np.concatenate(outs, axis=0).astype(np.float32)

